# revision 3
# baseline (speedup 1.0000x reference)
"""Trainium2 Bass kernel for nn_DigitConvolutionalModel.

Model: out = relu(conv2d_valid(x.reshape(28,28), conv_w).reshape(676) @ w1 + b1) @ w2 + b2

Strategy:
  - The 3x3 valid conv is a linear map C [784, 676]; fold it into the first
    FC layer on the host: W1' = C @ w1  [784, 300]. The device then runs a
    plain 2-layer MLP: out = relu(x @ W1' + b1) @ w2 + b2.
  - Pure data parallel over 8 NeuronCores: batch 65536 -> 8192 per core.
  - Feature-major device layout: host supplies x.T per core so the
    contraction dim sits on SBUF partitions for both matmul operands.
    Layer 1 computes g = relu(W1'.T @ x.T + b1) as [300, batch]; layer 2
    reuses g as the moving operand: out.T = w2.T @ g + b2 [10, batch].
  - PE-array quadrant packing (the streaming cost of a matmul is its moving
    column count, independent of the stationary size, and matmuls in
    disjoint row/col groups of the 128x128 array run concurrently):
      * K = 784 = 6 full 128-chunks + a 16-row remainder. The remainder for
        all four (m-chunk, batch-half) accumulators runs as 4 concurrent
        row-tiled matmuls at tile_position (0,0)/(32,0)/(64,0)/(96,0); the
        host pre-replicates the 16 remainder rows of x and W1' at
        partitions 0/32/64/96.
      * M = 300 = 2 full chunks + 44-col m2 chunk. m2 runs both batch
        halves concurrently col-tiled at (0,0)/(0,64); its K-remainder
        corner is a 2-way quadrant slot (0,0)/(32,64).
      * Layer 2 (M=10) runs both batch halves concurrently col-tiled into
        partitions 0..9 / 64..73 of one PSUM bank.
    Per 1024-col batch pair: 24 + 6 + 1 + 1 + 3 = 35 sequential PE slots
    of 512 moving columns vs 41 for the naive chunking.
  - bf16 matmul inputs (1 PE cycle/column vs 4 for fp32), fp32 PSUM
    accumulate. Remainder slots carry the accumulation stops and sit at
    the pair end so PSUM buffer rotation matches the baseline pipeline.
  - Layer 2 for pair p is software-pipelined into pair p+1's k-sweep.
  - Warm-up matmuls bridge the DMA prologue so the PE HAM clock gate is
    released (2.4 GHz) by the time the real stream starts.
"""

import numpy as np
import ml_dtypes

_B = 65536
_NCORES = 8
_BSH = _B // _NCORES  # 8192 batch rows per core
_N = 512  # batch columns per matmul (one fp32 PSUM bank)
_NK = 6  # full 128-row K chunks (784 = 6*128 + 16)
_KR = 16  # K remainder rows
_MP = 384  # padded hidden features for b1/w2 chunking (300 -> 3 chunks)
_M2 = 300 - 256  # 44 real rows of the third m-chunk
_NPAIR = _BSH // (2 * _N)  # 8 pairs of 512-col batch tiles
_NWARM = 26

_state = {}


def _build_nc():
    import concourse.tile as tile
    from concourse import bacc, mybir
    from contextlib import ExitStack

    dt = mybir.dt
    AF = mybir.ActivationFunctionType

    nc = bacc.Bacc(
        "TRN2",
        target_bir_lowering=False,
        debug=False,
        enable_asserts=False,
        num_devices=_NCORES,
    )

    xt = nc.dram_tensor("xt", [_NK * 128, _BSH], dt.bfloat16, kind="ExternalInput").ap()
    xrem = nc.dram_tensor("xrem", [128, _BSH], dt.bfloat16, kind="ExternalInput").ap()
    w1 = nc.dram_tensor("w1", [_NK * 128, _MP], dt.bfloat16, kind="ExternalInput").ap()
    w1rem = nc.dram_tensor("w1rem", [128, _MP], dt.bfloat16, kind="ExternalInput").ap()
    b1 = nc.dram_tensor("b1", [_MP, 1], dt.float32, kind="ExternalInput").ap()
    w2 = nc.dram_tensor("w2", [_MP, 10], dt.bfloat16, kind="ExternalInput").ap()
    b2 = nc.dram_tensor("b2", [10, 1], dt.float32, kind="ExternalInput").ap()
    outT = nc.dram_tensor("outT", [10, _BSH], dt.float32, kind="ExternalOutput").ap()

    # Partition-chunked DRAM views: [(chunk, p), cols] -> [p, chunk, cols]
    xt_r = xt.rearrange("(k p) c -> p k c", p=128)  # [128, 6, 8192]
    w1_r = w1.rearrange("(k p) m -> p k m", p=128)  # [128, 6, 384]
    b1_r = b1.rearrange("(m p) one -> p m one", p=128)  # [128, 3, 1]
    w2_r = w2.rearrange("(m p) o -> p m o", p=128)  # [128, 3, 10]

    with tile.TileContext(nc) as tc, ExitStack() as ctx:
        wpool = ctx.enter_context(tc.tile_pool(name="wpool", bufs=1))
        xpool = ctx.enter_context(tc.tile_pool(name="xpool", bufs=3))
        xrpool = ctx.enter_context(tc.tile_pool(name="xrpool", bufs=3))
        gpool = ctx.enter_context(tc.tile_pool(name="gpool", bufs=2))
        ppool = ctx.enter_context(tc.tile_pool(name="ppool", bufs=5, space="PSUM"))
        pm2pool = ctx.enter_context(tc.tile_pool(name="pm2pool", bufs=1, space="PSUM"))
        p2pool = ctx.enter_context(tc.tile_pool(name="p2pool", bufs=2, space="PSUM"))
        opool = ctx.enter_context(tc.tile_pool(name="opool", bufs=2))

        # PE warm-up: dependency-free matmuls on a zeroed scratch tile keep the
        # TensorEngine busy through the HAM activity window while the first
        # real DMAs land, so the real matmul stream starts at 2.4 GHz.
        warm_in = wpool.tile([128, 128], dt.bfloat16, name="warm_in", tag="warm_in")
        nc.gpsimd.memset(warm_in[:], 0.0)
        warm_ps = p2pool.tile([128, 128], dt.float32, name="warm_ps", tag="ps2")
        for _ in range(_NWARM):
            nc.tensor.matmul(
                out=warm_ps[:], lhsT=warm_in[:], rhs=warm_in[:], start=True, stop=True
            )

        # Stationary weights + first batch pair, interleaved per k-chunk so the
        # first matmul only waits on its own (w1[k0], xt[k0]) slices. DMA
        # *issue* costs ~0.7us/call of descriptor generation on the issuing
        # engine, so the prologue spreads issues across three otherwise-idle
        # engine queues: weights on sync, pair-0 x on gpsimd, pair-1 x +
        # params on scalar (ACT idle until ~16us).
        w1sb = wpool.tile([128, _NK, _MP], dt.bfloat16, name="w1sb", tag="w1sb")
        xt0 = xpool.tile([128, _NK, 2 * _N], dt.bfloat16, name="xt_0", tag="xt")
        for ki in range(_NK):
            nc.sync.dma_start(out=w1sb[:, ki, :], in_=w1_r[:, ki, :])
            nc.gpsimd.dma_start(out=xt0[:, ki, :], in_=xt_r[:, ki, 0 : 2 * _N])
        w1rsb = wpool.tile([128, _MP], dt.bfloat16, name="w1rsb", tag="w1rsb")
        nc.sync.dma_start(out=w1rsb[:], in_=w1rem[:, :])
        xr0 = xrpool.tile([128, 2 * _N], dt.bfloat16, name="xr_0", tag="xr")
        nc.gpsimd.dma_start(out=xr0[:], in_=xrem[:, 0 : 2 * _N])

        b1sb = wpool.tile([128, 3, 1], dt.float32, name="b1sb", tag="b1sb")
        nc.scalar.dma_start(out=b1sb[:], in_=b1_r[:])
        w2sb = wpool.tile([128, 3, 10], dt.bfloat16, name="w2sb", tag="w2sb")
        nc.scalar.dma_start(out=w2sb[:], in_=w2_r[:])
        b2sb = wpool.tile([10, 1], dt.float32, name="b2sb", tag="b2sb")
        nc.scalar.dma_start(out=b2sb[:], in_=b2[:, :])
        # Batch-half j=1 lands at psum/sbuf partitions 64.. so its biases and
        # layer-2 m2 weights need partition-64-aligned replicas.
        b1rep = wpool.tile([128, 1], dt.float32, name="b1rep", tag="b1rep")
        nc.scalar.dma_start(out=b1rep[64 : 64 + _M2, :], in_=b1_r[0:_M2, 2, :])
        b2rep = wpool.tile([128, 1], dt.float32, name="b2rep", tag="b2rep")
        nc.scalar.dma_start(out=b2rep[64:74, :], in_=b2[:, :])
        w2rep = wpool.tile([128, 10], dt.bfloat16, name="w2rep", tag="w2rep")
        nc.gpsimd.memset(w2rep[:], 0.0)
        nc.scalar.dma_start(out=w2rep[64 : 64 + _M2, :], in_=w2_r[0:_M2, 2, :])
        # Pair 1's x on scalar so sync/gpsimd stay free for steady-state.
        xt1 = xpool.tile([128, _NK, 2 * _N], dt.bfloat16, name="xt_1", tag="xt")
        nc.scalar.dma_start(out=xt1[:], in_=xt_r[:, :, 2 * _N : 4 * _N])
        xr1 = xrpool.tile([128, 2 * _N], dt.bfloat16, name="xr_1", tag="xr")
        nc.scalar.dma_start(out=xr1[:], in_=xrem[:, 2 * _N : 4 * _N])

        # Persistent zero-padded g tiles for the m2 chunk (44 real rows), two
        # per batch half, alternated by pair parity. The zero rows make layer
        # 2's m2 matmul a uniform full-128-row matmul (0-weight x 0-value).
        # Memset once; each pair's relu rewrites only the real rows.
        g2t = {}
        for j, base in ((0, 0), (1, 64)):
            for par in range(2):
                g = wpool.tile([128, _N], dt.bfloat16, name=f"g2_{j}_{par}", tag=f"g2_{j}_{par}")
                if j == 0:
                    nc.gpsimd.memset(g[32:64, :], 0.0)  # relu rewrites 32..43
                    nc.gpsimd.memset(g[64:128, :], 0.0)
                else:
                    nc.gpsimd.memset(g[0:64, :], 0.0)
                    nc.gpsimd.memset(g[96:128, :], 0.0)  # relu rewrites 96..107
                g2t[(j, par)] = g

        def layer2(prev_g, prev_c0):
            """Second layer for the pair at column prev_c0: both batch halves
            run concurrently as col-tiled matmuls into one PSUM bank
            (j=0 -> partitions 0..9, j=1 -> partitions 64..73)."""
            ps2 = p2pool.tile([128, _N], dt.float32, name=f"ps2_{prev_c0}", tag="ps2")
            for mi in range(3):
                for j in range(2):
                    w = w2sb[:, mi, :] if (mi < 2 or j == 0) else w2rep[:]
                    nc.tensor.matmul(
                        out=ps2[64 * j : 64 * j + 10, :],
                        lhsT=w,
                        rhs=prev_g[(mi, j)][:],
                        start=(mi == 0),
                        stop=(mi == 2),
                        tile_position=(0, 64 * j),
                    )
            ob = opool.tile([128, _N], dt.float32, name=f"ob_{prev_c0}", tag="ob")
            # Split the psum drains across ACT and DVE.
            nc.scalar.activation(ob[0:10, :], ps2[0:10, :], AF.Identity, bias=b2sb[:], scale=1.0)
            nc.vector.tensor_scalar(ob[64:74, :], ps2[64:74, :], b2rep[64:74, :], None, mybir.AluOpType.add)
            nc.sync.dma_start(out=outT[:, prev_c0 : prev_c0 + _N], in_=ob[0:10, :])
            nc.sync.dma_start(out=outT[:, prev_c0 + _N : prev_c0 + 2 * _N], in_=ob[64:74, :])

        prev_g = None
        prev_c0 = 0
        for pair in range(_NPAIR):
            c0 = pair * 2 * _N
            if pair == 0:
                xtile, xr = xt0, xr0
            elif pair == 1:
                xtile, xr = xt1, xr1
            else:
                xtile = xpool.tile([128, _NK, 2 * _N], dt.bfloat16, name=f"xt_{pair}", tag="xt")
                nc.sync.dma_start(out=xtile[:], in_=xt_r[:, :, c0 : c0 + 2 * _N])
                xr = xrpool.tile([128, 2 * _N], dt.bfloat16, name=f"xr_{pair}", tag="xr")
                nc.gpsimd.dma_start(out=xr[:], in_=xrem[:, c0 : c0 + 2 * _N])

            cur_g = {}
            ps = {
                (mi, j): ppool.tile([128, _N], dt.float32, name=f"ps_{pair}_{mi}_{j}", tag="ps")
                for mi in range(2)
                for j in range(2)
            }
            psm2 = pm2pool.tile([128, _N], dt.float32, name=f"psm2_{pair}", tag="psm2")
            # k-major sweep: consume each 1.5MB/6 x-chunk for all four main
            # accumulators plus the col-tiled m2 pair (~1.1us of matmul)
            # before needing the next chunk (~0.8us DMA delivery).
            for ki in range(_NK):
                for mi in range(2):
                    for j in range(2):
                        nc.tensor.matmul(
                            out=ps[(mi, j)][:],
                            lhsT=w1sb[:, ki, mi * 128 : (mi + 1) * 128],
                            rhs=xtile[:, ki, j * _N : (j + 1) * _N],
                            start=(ki == 0),
                            stop=False,
                        )
                # m2 chunk: both batch halves concurrently col-tiled (j=0 ->
                # psum partitions 0..43, j=1 -> partitions 64..107).
                for j in range(2):
                    nc.tensor.matmul(
                        out=psm2[64 * j : 64 * j + _M2, :],
                        lhsT=w1sb[:, ki, 256 : 256 + _M2],
                        rhs=xtile[:, ki, j * _N : (j + 1) * _N],
                        start=(ki == 0),
                        stop=False,
                        tile_position=(0, 64 * j),
                    )
                if ki == 2 and prev_g is not None:
                    # Software-pipelined layer 2 for the previous pair.
                    layer2(prev_g, prev_c0)

            # K-remainder (rows 768..783): all four main accumulators in ONE
            # slot as 4 concurrent row-tiled matmuls; x/w1 remainder rows are
            # host-replicated at partitions 0/32/64/96. Carries the stops.
            for (mi, j), row in (((0, 0), 0), ((1, 0), 32), ((0, 1), 64), ((1, 1), 96)):
                nc.tensor.matmul(
                    out=ps[(mi, j)][:],
                    lhsT=w1rsb[row : row + _KR, mi * 128 : (mi + 1) * 128],
                    rhs=xr[row : row + _KR, j * _N : (j + 1) * _N],
                    start=False,
                    stop=True,
                    tile_position=(row, 0),
                )
            # m2 K-remainder corner: 2-way quadrant slot.
            nc.tensor.matmul(
                out=psm2[0:_M2, :],
                lhsT=w1rsb[0:_KR, 256 : 256 + _M2],
                rhs=xr[0:_KR, 0:_N],
                start=False,
                stop=True,
                tile_position=(0, 0),
            )
            nc.tensor.matmul(
                out=psm2[64 : 64 + _M2, :],
                lhsT=w1rsb[32 : 32 + _KR, 256 : 256 + _M2],
                rhs=xr[32 : 32 + _KR, _N : 2 * _N],
                start=False,
                stop=True,
                tile_position=(32, 64),
            )

            for mi in range(2):
                for j in range(2):
                    g = gpool.tile([128, _N], dt.bfloat16, name=f"g_{pair}_{mi}_{j}", tag=f"g{mi}{j}")
                    if j == 0:
                        # Split the relus across ACT and DVE so neither engine
                        # serializes the psum drain.
                        nc.scalar.activation(
                            g[:], ps[(mi, j)][:], AF.Relu, bias=b1sb[:, mi, :], scale=1.0
                        )
                    else:
                        nc.vector.tensor_scalar(
                            g[:], ps[(mi, j)][:], b1sb[:, mi, :], 0.0,
                            mybir.AluOpType.add, mybir.AluOpType.max,
                        )
                    cur_g[(mi, j)] = g
            g20 = g2t[(0, pair % 2)]
            nc.scalar.activation(
                g20[0:_M2, :], psm2[0:_M2, :], AF.Relu, bias=b1sb[0:_M2, 2, :], scale=1.0
            )
            g21 = g2t[(1, pair % 2)]
            nc.vector.tensor_scalar(
                g21[64 : 64 + _M2, :], psm2[64 : 64 + _M2, :], b1rep[64 : 64 + _M2, :],
                0.0, mybir.AluOpType.add, mybir.AluOpType.max,
            )
            cur_g[(2, 0)] = g20
            cur_g[(2, 1)] = g21
            prev_g = cur_g
            prev_c0 = c0
        layer2(prev_g, prev_c0)

    nc.compile()
    return nc


def _fold_conv(conv_w, w1):
    """W1' = C @ w1 where C [784, 676] is the linear map of the 3x3 valid conv."""
    C = np.zeros((784, 676), np.float64)
    cw = np.asarray(conv_w, np.float64)
    for di in range(3):
        for dj in range(3):
            for i in range(26):
                rows = (i + di) * 28 + dj + np.arange(26)
                C[rows, i * 26 + np.arange(26)] += cw[di, dj]
    return C @ np.asarray(w1, np.float64)  # [784, 300]


def _exec(inputs, trace=False, **run_kwargs):
    from concourse.bass_utils import run_bass_kernel_spmd

    x = np.asarray(inputs["x"], np.float32)
    bf16 = ml_dtypes.bfloat16

    w1full = _fold_conv(inputs["conv_w"], inputs["w1"])  # [784, 300] f64
    w1main = np.zeros((_NK * 128, _MP), bf16)
    w1main[:, :300] = w1full[: _NK * 128].astype(bf16)
    w1remf = np.zeros((128, _MP), bf16)
    for off in (0, 32, 64, 96):
        w1remf[off : off + _KR, :300] = w1full[_NK * 128 :].astype(bf16)
    b1c = np.zeros((_MP, 1), np.float32)
    b1c[:300, 0] = np.asarray(inputs["b1"], np.float32)
    w2b = np.zeros((_MP, 10), bf16)
    w2b[:300] = np.asarray(inputs["w2"], np.float32).astype(bf16)
    b2c = np.ascontiguousarray(np.asarray(inputs["b2"], np.float32).reshape(10, 1))

    if "nc" not in _state:
        _state["nc"] = _build_nc()
    nc = _state["nc"]

    xb = x.astype(bf16)  # [65536, 784]
    in_maps = []
    for c in range(_NCORES):
        blk = xb[c * _BSH : (c + 1) * _BSH, :]  # [8192, 784]
        xtc = np.ascontiguousarray(blk[:, : _NK * 128].T)  # [768, 8192]
        tail = blk[:, _NK * 128 :].T  # [16, 8192]
        xrc = np.zeros((128, _BSH), bf16)
        for off in (0, 32, 64, 96):
            xrc[off : off + _KR] = tail
        in_maps.append(
            {"xt": xtc, "xrem": xrc, "w1": w1main, "w1rem": w1remf,
             "b1": b1c, "w2": w2b, "b2": b2c}
        )

    res = run_bass_kernel_spmd(
        nc, in_maps, list(range(_NCORES)), trace=trace, **run_kwargs
    )
    outs = [res.results[c]["outT"] for c in range(_NCORES)]  # each [10, 8192]
    out = np.concatenate(outs, axis=1).T  # [65536, 10]
    return np.ascontiguousarray(out, dtype=np.float32), res


def kernel(**inputs):
    out, _ = _exec(inputs, trace=False)
    return out


# revision 6
# speedup vs baseline: 1.1509x; 1.1509x over previous
"""Trainium2 Bass kernel for nn_DigitConvolutionalModel.

Model: out = relu(conv2d_valid(x.reshape(28,28), conv_w).reshape(676) @ w1 + b1) @ w2 + b2

Strategy:
  - The 3x3 valid conv is a linear map C [784, 676]; fold it into the first
    FC layer on the host: W1' = C @ w1  [784, 300]. The device then runs a
    plain 2-layer MLP: out = relu(x @ W1' + b1) @ w2 + b2.
  - Pure data parallel over 8 NeuronCores: batch 65536 -> 8192 per core.
  - Feature-major device layout: host supplies x.T per core so the
    contraction dim sits on SBUF partitions for both matmul operands.
    Layer 1 computes g = relu(W1'.T @ x.T + b1) as [300, batch]; layer 2
    reuses g as the moving operand: out.T = w2.T @ g + b2 [10, batch].
  - PE-array quadrant packing (the streaming cost of a matmul is its moving
    column count, independent of the stationary size, and matmuls in
    disjoint row/col groups of the 128x128 array run concurrently):
      * K = 784 = 6 full 128-chunks + a 16-row remainder. The remainder for
        all four (m-chunk, batch-half) accumulators runs as 4 concurrent
        row-tiled matmuls at tile_position (0,0)/(32,0)/(64,0)/(96,0); the
        host pre-replicates the 16 remainder rows of x and W1' at
        partitions 0/32/64/96.
      * M = 300 = 2 full chunks + 44-col m2 chunk. m2 runs both batch
        halves concurrently col-tiled at (0,0)/(0,64); its K-remainder
        corner is a 2-way quadrant slot (0,0)/(32,64).
      * Layer 2 (M=10) runs both batch halves concurrently col-tiled into
        partitions 0..9 / 64..73 of one PSUM bank.
    Per 1024-col batch pair: 24 + 6 + 1 + 1 + 3 = 35 sequential PE slots
    of 512 moving columns vs 41 for the naive chunking.
  - bf16 matmul inputs (1 PE cycle/column vs 4 for fp32), fp32 PSUM
    accumulate. Remainder slots carry the accumulation stops and sit at
    the pair end so PSUM buffer rotation matches the baseline pipeline.
  - Layer 2 for pair p is software-pipelined into pair p+1's k-sweep.
  - Warm-up matmuls bridge the DMA prologue so the PE HAM clock gate is
    released (2.4 GHz) by the time the real stream starts.
"""

import numpy as np
import ml_dtypes

_B = 65536
_NCORES = 8
_BSH = _B // _NCORES  # 8192 batch rows per core
_N = 512  # batch columns per matmul (one fp32 PSUM bank)
_NK = 6  # full 128-row K chunks (784 = 6*128 + 16)
_KR = 16  # K remainder rows
_MP = 384  # padded hidden features for b1/w2 chunking (300 -> 3 chunks)
_M2 = 300 - 256  # 44 real rows of the third m-chunk
_NPAIR = _BSH // (2 * _N)  # 8 pairs of 512-col batch tiles
_NWARM = 26

_state = {}


def _build_nc():
    import concourse.tile as tile
    from concourse import bacc, mybir
    from contextlib import ExitStack

    dt = mybir.dt
    AF = mybir.ActivationFunctionType

    nc = bacc.Bacc(
        "TRN2",
        target_bir_lowering=False,
        debug=False,
        enable_asserts=False,
        num_devices=_NCORES,
    )

    xt = nc.dram_tensor("xt", [_NK * 128, _BSH], dt.bfloat16, kind="ExternalInput").ap()
    xrem = nc.dram_tensor("xrem", [128, _BSH], dt.bfloat16, kind="ExternalInput").ap()
    w1 = nc.dram_tensor("w1", [_NK * 128, _MP], dt.bfloat16, kind="ExternalInput").ap()
    w1rem = nc.dram_tensor("w1rem", [128, _MP], dt.bfloat16, kind="ExternalInput").ap()
    b1 = nc.dram_tensor("b1", [_MP, 1], dt.float32, kind="ExternalInput").ap()
    w2 = nc.dram_tensor("w2", [_MP, 10], dt.bfloat16, kind="ExternalInput").ap()
    b2 = nc.dram_tensor("b2", [10, 1], dt.float32, kind="ExternalInput").ap()
    outT = nc.dram_tensor("outT", [10, _BSH], dt.float32, kind="ExternalOutput").ap()

    # Partition-chunked DRAM views: [(chunk, p), cols] -> [p, chunk, cols]
    xt_r = xt.rearrange("(k p) c -> p k c", p=128)  # [128, 6, 8192]
    w1_r = w1.rearrange("(k p) m -> p k m", p=128)  # [128, 6, 384]
    b1_r = b1.rearrange("(m p) one -> p m one", p=128)  # [128, 3, 1]
    w2_r = w2.rearrange("(m p) o -> p m o", p=128)  # [128, 3, 10]

    with tile.TileContext(nc) as tc, ExitStack() as ctx:
        wpool = ctx.enter_context(tc.tile_pool(name="wpool", bufs=1))
        xpool = ctx.enter_context(tc.tile_pool(name="xpool", bufs=3))
        xrpool = ctx.enter_context(tc.tile_pool(name="xrpool", bufs=3))
        gpool = ctx.enter_context(tc.tile_pool(name="gpool", bufs=2))
        ppool = ctx.enter_context(tc.tile_pool(name="ppool", bufs=5, space="PSUM"))
        pm2pool = ctx.enter_context(tc.tile_pool(name="pm2pool", bufs=1, space="PSUM"))
        p2pool = ctx.enter_context(tc.tile_pool(name="p2pool", bufs=2, space="PSUM"))
        opool = ctx.enter_context(tc.tile_pool(name="opool", bufs=2))

        # PE warm-up: dependency-free matmuls on a zeroed scratch tile keep the
        # TensorEngine busy through the HAM activity window while the first
        # real DMAs land, so the real matmul stream starts at 2.4 GHz.
        warm_in = wpool.tile([128, 128], dt.bfloat16, name="warm_in", tag="warm_in")
        nc.gpsimd.memset(warm_in[:], 0.0)
        warm_ps = p2pool.tile([128, 128], dt.float32, name="warm_ps", tag="ps2")
        for _ in range(_NWARM):
            nc.tensor.matmul(
                out=warm_ps[:], lhsT=warm_in[:], rhs=warm_in[:], start=True, stop=True
            )

        # Stationary weights + first batch pair, interleaved per k-chunk so the
        # first matmul only waits on its own (w1[k0], xt[k0]) slices. DMA
        # *issue* costs ~0.7us/call of descriptor generation on the issuing
        # engine, so the prologue spreads issues across three otherwise-idle
        # engine queues: weights on sync, pair-0 x on gpsimd, pair-1 x +
        # params on scalar (ACT idle until ~16us).
        w1sb = wpool.tile([128, _NK, _MP], dt.bfloat16, name="w1sb", tag="w1sb")
        xt0 = xpool.tile([128, _NK, 2 * _N], dt.bfloat16, name="xt_0", tag="xt")
        for ki in range(_NK):
            nc.sync.dma_start(out=w1sb[:, ki, :], in_=w1_r[:, ki, :])
            nc.gpsimd.dma_start(out=xt0[:, ki, :], in_=xt_r[:, ki, 0 : 2 * _N])
        w1rsb = wpool.tile([128, _MP], dt.bfloat16, name="w1rsb", tag="w1rsb")
        nc.sync.dma_start(out=w1rsb[:], in_=w1rem[:, :])
        xr0 = xrpool.tile([128, 2 * _N], dt.bfloat16, name="xr_0", tag="xr")
        nc.gpsimd.dma_start(out=xr0[:], in_=xrem[:, 0 : 2 * _N])

        b1sb = wpool.tile([128, 3, 1], dt.float32, name="b1sb", tag="b1sb")
        nc.scalar.dma_start(out=b1sb[:], in_=b1_r[:])
        w2sb = wpool.tile([128, 3, 10], dt.bfloat16, name="w2sb", tag="w2sb")
        nc.scalar.dma_start(out=w2sb[:], in_=w2_r[:])
        b2sb = wpool.tile([10, 1], dt.float32, name="b2sb", tag="b2sb")
        nc.scalar.dma_start(out=b2sb[:], in_=b2[:, :])
        # Batch-half j=1 lands at psum/sbuf partitions 64.. so its biases and
        # layer-2 m2 weights need partition-64-aligned replicas.
        b1rep = wpool.tile([128, 1], dt.float32, name="b1rep", tag="b1rep")
        nc.scalar.dma_start(out=b1rep[64 : 64 + _M2, :], in_=b1_r[0:_M2, 2, :])
        b2rep = wpool.tile([128, 1], dt.float32, name="b2rep", tag="b2rep")
        nc.scalar.dma_start(out=b2rep[64:74, :], in_=b2[:, :])
        w2rep = wpool.tile([128, 10], dt.bfloat16, name="w2rep", tag="w2rep")
        nc.gpsimd.memset(w2rep[:], 0.0)
        nc.scalar.dma_start(out=w2rep[64 : 64 + _M2, :], in_=w2_r[0:_M2, 2, :])
        # Pair 1's x on scalar and pair 2's on vector so sync/gpsimd stay free
        # for steady-state and the early pairs are already in flight while
        # pair 0 computes.
        xt1 = xpool.tile([128, _NK, 2 * _N], dt.bfloat16, name="xt_1", tag="xt")
        nc.scalar.dma_start(out=xt1[:], in_=xt_r[:, :, 2 * _N : 4 * _N])
        xr1 = xrpool.tile([128, 2 * _N], dt.bfloat16, name="xr_1", tag="xr")
        nc.scalar.dma_start(out=xr1[:], in_=xrem[:, 2 * _N : 4 * _N])
        xt2 = xpool.tile([128, _NK, 2 * _N], dt.bfloat16, name="xt_2", tag="xt")
        nc.scalar.dma_start(out=xt2[:, 0:3, :], in_=xt_r[:, 0:3, 4 * _N : 6 * _N])
        nc.scalar.dma_start(out=xt2[:, 3:6, :], in_=xt_r[:, 3:6, 4 * _N : 6 * _N])
        xr2 = xrpool.tile([128, 2 * _N], dt.bfloat16, name="xr_2", tag="xr")
        nc.scalar.dma_start(out=xr2[:], in_=xrem[:, 4 * _N : 6 * _N])

        # Persistent zero-padded g tiles for the m2 chunk (44 real rows), two
        # per batch half, alternated by pair parity. The zero rows make layer
        # 2's m2 matmul a uniform full-128-row matmul (0-weight x 0-value).
        # Memset once; each pair's relu rewrites only the real rows.
        g2t = {}
        for j, base in ((0, 0), (1, 64)):
            for par in range(2):
                g = wpool.tile([128, _N], dt.bfloat16, name=f"g2_{j}_{par}", tag=f"g2_{j}_{par}")
                if j == 0:
                    nc.gpsimd.memset(g[32:64, :], 0.0)  # relu rewrites 32..43
                    nc.gpsimd.memset(g[64:128, :], 0.0)
                else:
                    nc.gpsimd.memset(g[0:64, :], 0.0)
                    nc.gpsimd.memset(g[96:128, :], 0.0)  # relu rewrites 96..107
                g2t[(j, par)] = g

        def layer2(prev_g, prev_c0):
            """Second layer for the pair at column prev_c0: both batch halves
            run concurrently as col-tiled matmuls into one PSUM bank
            (j=0 -> partitions 0..9, j=1 -> partitions 64..73)."""
            ps2 = p2pool.tile([128, _N], dt.float32, name=f"ps2_{prev_c0}", tag="ps2")
            for mi in range(3):
                for j in range(2):
                    w = w2sb[:, mi, :] if (mi < 2 or j == 0) else w2rep[:]
                    nc.tensor.matmul(
                        out=ps2[64 * j : 64 * j + 10, :],
                        lhsT=w,
                        rhs=prev_g[(mi, j)][:],
                        start=(mi == 0),
                        stop=(mi == 2),
                        tile_position=(0, 64 * j),
                    )
            ob = opool.tile([128, _N], dt.float32, name=f"ob_{prev_c0}", tag="ob")
            # Split the psum drains across ACT and DVE.
            nc.scalar.activation(ob[0:10, :], ps2[0:10, :], AF.Identity, bias=b2sb[:], scale=1.0)
            nc.vector.tensor_scalar(ob[64:74, :], ps2[64:74, :], b2rep[64:74, :], None, mybir.AluOpType.add)
            nc.sync.dma_start(out=outT[:, prev_c0 : prev_c0 + _N], in_=ob[0:10, :])
            nc.sync.dma_start(out=outT[:, prev_c0 + _N : prev_c0 + 2 * _N], in_=ob[64:74, :])

        prev_g = None
        prev_c0 = 0
        for pair in range(_NPAIR):
            c0 = pair * 2 * _N
            if pair == 0:
                xtile, xr = xt0, xr0
            elif pair == 1:
                xtile, xr = xt1, xr1
            elif pair == 2:
                xtile, xr = xt2, xr2
            else:
                xtile = xpool.tile([128, _NK, 2 * _N], dt.bfloat16, name=f"xt_{pair}", tag="xt")
                nc.sync.dma_start(out=xtile[:], in_=xt_r[:, :, c0 : c0 + 2 * _N])
                xr = xrpool.tile([128, 2 * _N], dt.bfloat16, name=f"xr_{pair}", tag="xr")
                nc.gpsimd.dma_start(out=xr[:], in_=xrem[:, c0 : c0 + 2 * _N])

            cur_g = {}
            ps = {
                (mi, j): ppool.tile([128, _N], dt.float32, name=f"ps_{pair}_{mi}_{j}", tag="ps")
                for mi in range(2)
                for j in range(2)
            }
            psm2 = pm2pool.tile([128, _N], dt.float32, name=f"psm2_{pair}", tag="psm2")
            # Same-mode matmuls are grouped into blocks: switching the PE
            # between full-array and row/col-tiled modes costs a pipeline
            # drain, so per-ki interleaving of modes is ~20% slower.
            # Block 1: main accumulators, 24 full-array matmuls, k-major so
            # each 256KB x-chunk is consumed ~0.9us after use starts while
            # the next chunk needs ~0.8us of DMA delivery.
            for ki in range(_NK):
                for mi in range(2):
                    for j in range(2):
                        nc.tensor.matmul(
                            out=ps[(mi, j)][:],
                            lhsT=w1sb[:, ki, mi * 128 : (mi + 1) * 128],
                            rhs=xtile[:, ki, j * _N : (j + 1) * _N],
                            start=(ki == 0),
                            stop=False,
                        )
            # Block 2 (one slot): K-remainder rows 768..783 for all four main
            # accumulators as 4 concurrent row-tiled matmuls; x/w1 remainder
            # rows are host-replicated at partitions 0/32/64/96. Carries the
            # stops so the relus can start while blocks 3-4 stream.
            for (mi, j), row in (((0, 0), 0), ((1, 0), 32), ((0, 1), 64), ((1, 1), 96)):
                nc.tensor.matmul(
                    out=ps[(mi, j)][:],
                    lhsT=w1rsb[row : row + _KR, mi * 128 : (mi + 1) * 128],
                    rhs=xr[row : row + _KR, j * _N : (j + 1) * _N],
                    start=False,
                    stop=True,
                    tile_position=(row, 0),
                )
            for mi in range(2):
                for j in range(2):
                    g = gpool.tile([128, _N], dt.bfloat16, name=f"g_{pair}_{mi}_{j}", tag=f"g{mi}{j}")
                    if j == 0:
                        # Split the relus across ACT and DVE so neither engine
                        # serializes the psum drain.
                        nc.scalar.activation(
                            g[:], ps[(mi, j)][:], AF.Relu, bias=b1sb[:, mi, :], scale=1.0
                        )
                    else:
                        nc.vector.tensor_scalar(
                            g[:], ps[(mi, j)][:], b1sb[:, mi, :], 0.0,
                            mybir.AluOpType.add, mybir.AluOpType.max,
                        )
                    cur_g[(mi, j)] = g
            # Block 3: software-pipelined layer 2 for the previous pair
            # (col-tiled, keeps the PE in a tiled mode between blocks 2 and 4).
            if prev_g is not None:
                layer2(prev_g, prev_c0)
            # Block 4: m2 chunk, both batch halves concurrently col-tiled
            # (j=0 -> psum partitions 0..43, j=1 -> partitions 64..107), then
            # its K-remainder corner as a 2-way quadrant slot.
            for ki in range(_NK):
                for j in range(2):
                    nc.tensor.matmul(
                        out=psm2[64 * j : 64 * j + _M2, :],
                        lhsT=w1sb[:, ki, 256 : 256 + _M2],
                        rhs=xtile[:, ki, j * _N : (j + 1) * _N],
                        start=(ki == 0),
                        stop=False,
                        tile_position=(0, 64 * j),
                    )
            nc.tensor.matmul(
                out=psm2[0:_M2, :],
                lhsT=w1rsb[0:_KR, 256 : 256 + _M2],
                rhs=xr[0:_KR, 0:_N],
                start=False,
                stop=True,
                tile_position=(0, 0),
            )
            nc.tensor.matmul(
                out=psm2[64 : 64 + _M2, :],
                lhsT=w1rsb[32 : 32 + _KR, 256 : 256 + _M2],
                rhs=xr[32 : 32 + _KR, _N : 2 * _N],
                start=False,
                stop=True,
                tile_position=(32, 64),
            )
            g20 = g2t[(0, pair % 2)]
            nc.scalar.activation(
                g20[0:_M2, :], psm2[0:_M2, :], AF.Relu, bias=b1sb[0:_M2, 2, :], scale=1.0
            )
            g21 = g2t[(1, pair % 2)]
            nc.vector.tensor_scalar(
                g21[64 : 64 + _M2, :], psm2[64 : 64 + _M2, :], b1rep[64 : 64 + _M2, :],
                0.0, mybir.AluOpType.add, mybir.AluOpType.max,
            )
            cur_g[(2, 0)] = g20
            cur_g[(2, 1)] = g21
            prev_g = cur_g
            prev_c0 = c0
        layer2(prev_g, prev_c0)

    nc.compile()
    return nc


def _fold_conv(conv_w, w1):
    """W1' = C @ w1 where C [784, 676] is the linear map of the 3x3 valid conv."""
    C = np.zeros((784, 676), np.float64)
    cw = np.asarray(conv_w, np.float64)
    for di in range(3):
        for dj in range(3):
            for i in range(26):
                rows = (i + di) * 28 + dj + np.arange(26)
                C[rows, i * 26 + np.arange(26)] += cw[di, dj]
    return C @ np.asarray(w1, np.float64)  # [784, 300]


def _exec(inputs, trace=False, **run_kwargs):
    from concourse.bass_utils import run_bass_kernel_spmd

    x = np.asarray(inputs["x"], np.float32)
    bf16 = ml_dtypes.bfloat16

    w1full = _fold_conv(inputs["conv_w"], inputs["w1"])  # [784, 300] f64
    w1main = np.zeros((_NK * 128, _MP), bf16)
    w1main[:, :300] = w1full[: _NK * 128].astype(bf16)
    w1remf = np.zeros((128, _MP), bf16)
    for off in (0, 32, 64, 96):
        w1remf[off : off + _KR, :300] = w1full[_NK * 128 :].astype(bf16)
    b1c = np.zeros((_MP, 1), np.float32)
    b1c[:300, 0] = np.asarray(inputs["b1"], np.float32)
    w2b = np.zeros((_MP, 10), bf16)
    w2b[:300] = np.asarray(inputs["w2"], np.float32).astype(bf16)
    b2c = np.ascontiguousarray(np.asarray(inputs["b2"], np.float32).reshape(10, 1))

    if "nc" not in _state:
        _state["nc"] = _build_nc()
    nc = _state["nc"]

    xb = x.astype(bf16)  # [65536, 784]
    in_maps = []
    for c in range(_NCORES):
        blk = xb[c * _BSH : (c + 1) * _BSH, :]  # [8192, 784]
        xtc = np.ascontiguousarray(blk[:, : _NK * 128].T)  # [768, 8192]
        tail = blk[:, _NK * 128 :].T  # [16, 8192]
        xrc = np.zeros((128, _BSH), bf16)
        for off in (0, 32, 64, 96):
            xrc[off : off + _KR] = tail
        in_maps.append(
            {"xt": xtc, "xrem": xrc, "w1": w1main, "w1rem": w1remf,
             "b1": b1c, "w2": w2b, "b2": b2c}
        )

    res = run_bass_kernel_spmd(
        nc, in_maps, list(range(_NCORES)), trace=trace, **run_kwargs
    )
    outs = [res.results[c]["outT"] for c in range(_NCORES)]  # each [10, 8192]
    out = np.concatenate(outs, axis=1).T  # [65536, 10]
    return np.ascontiguousarray(out, dtype=np.float32), res


def kernel(**inputs):
    out, _ = _exec(inputs, trace=False)
    return out
